# revision 34
# baseline (speedup 1.0000x reference)
"""Bayesian categorical embedding lookup on 8 trn2 NeuronCores.

For each of 8 categorical columns: out = mu + softplus(rho) * eps gathered at
X[:, c]; outputs concatenated to [16384, 248] f32.

Strategy
  - Host packs each column's (mu, rho, eps) into one row-concatenated table so
    a single gathered row carries all three vectors.
  - Cols 0,1 (dim 64, 768B rows)  -> group A, vocab-sharded per column across
    the 8 cores (each core owns 1/8 of each column's rows, stacked); the host
    routes every (batch, col) pair to its owning core.
  - Cols 2,3 (dim 32, rows padded to 512B) -> group B, sharded + routed the
    same way.
  - Cols 4..7 (dims 16,16,16,8; rows padded to 256B) -> group CS, table
    replicated, batch-sharded (core k handles batch rows [2048k, 2048k+2048)).
  - Device per core: GPSIMD dma_gather (int16 indices; group A's 150002-row
    shard is gathered in 32768-row sub-ranges so indices fit int16), then
    softplus via Exp + Ln(x+1) on ACT, mult/add on DVE, compact stores.
  - Host scatters the routed rows back into the full output.

dma_gather contracts used here (see concourse/bass.py and bass_interp.py):
  - indices int16, element i at [i % 16, i // 16] of a [128, n/16] SBUF tile,
    that 16-row block replicated 8x down the partitions (one per Q7 core);
  - gathered row i lands at partition i % 128, slot i // 128 of the dst tile;
  - elem_size bytes must be a multiple of 256;
  - we pad every index segment with row 0 (always valid) so num_idxs is the
    same on all 8 cores (SPMD) and no -1 handling is needed.
"""

import numpy as np

N_CORES = 8
BATCH = 16384
BPC = BATCH // N_CORES  # 2048 batch rows per core

VOCABS = [1000000, 200000, 100000, 50000, 10000, 5000, 1000, 100]
NROWS = [v + 1 for v in VOCABS]
DIMS = [64, 64, 32, 32, 16, 16, 16, 8]
OFFS = [0, 64, 128, 160, 192, 208, 224, 240]
DTOT = 248

A_COLS, B_COLS, CS_COLS = (0, 1), (2, 3), (4, 5, 6, 7)
A_SH = [-(-NROWS[c] // N_CORES) for c in A_COLS]   # [125001, 25001]
S_A = sum(A_SH)                                    # 150002 rows per core
A_W = 192                                          # 768B f32 rows
SUB = 32768                                        # int16 sub-range size
A_RANGES = [(r, min(r + SUB, S_A)) for r in range(0, S_A, SUB)]
B_SH = [-(-NROWS[c] // N_CORES) for c in B_COLS]   # [12501, 6251]
S_B = sum(B_SH)                                    # 18752
B_W = 128                                          # 512B f32 rows
CS_BASE = [0]
for c in CS_COLS[:-1]:
    CS_BASE.append(CS_BASE[-1] + NROWS[c])
CS_ROWS = CS_BASE[-1] + NROWS[CS_COLS[-1]]         # 16104
CS_W = 64                                          # 256B rows
CS_N = BPC * len(CS_COLS)                          # 8192 gathered rows/core
CHUNK = 1024                                       # max idx per dma_gather
                                                   # (HW crashes above ~1024)


def _chunks(cap):
    return [(c0, min(c0 + CHUNK, cap)) for c0 in range(0, cap, CHUNK)]

_nc_cache = {}
last_result = None
RUN_MODE = "hw"  # "sim" runs CoreSim per core instead of hardware (debug)


def _build_nc(capsA, capB, softplus_native=True):
    """Build the SPMD Bacc program. capsA: rows gathered per A sub-range
    (each a multiple of 128, uniform across cores); capB likewise."""
    import concourse.bacc as bacc
    import concourse.mybir as mybir
    import concourse.tile as tile

    f32, i16 = mybir.dt.float32, mybir.dt.int16
    ACT = mybir.ActivationFunctionType
    ALU = mybir.AluOpType

    # Force Exp AND Ln onto the one ACT table containing both
    # (natural_log_exp_and_others): the table chooser otherwise alternates
    # exp_and_others <-> natural_log, reloading the table (1.28us) around
    # every chunk.  Table ids are dict positions, so only the function sets
    # are edited, never the order.
    if not getattr(bacc, "_ant_act_tables_patched", False):
        _orig_tables = bacc.get_activation_tables

        def _patched_tables(arch):
            t = dict(_orig_tables(arch))
            both = {mybir.ActivationFunctionType.Exp,
                    mybir.ActivationFunctionType.Ln}
            return {name: (fns if name == "natural_log_exp_and_others"
                           else fns - both)
                    for name, fns in t.items()}

        bacc.get_activation_tables = _patched_tables
        bacc._ant_act_tables_patched = True

    # 4 SWDGE queues: one qPoolDynamic ring throttles gather descriptor
    # flow to ~90 GB/s; round-robin over 4 rings measured ~1.75x faster.
    n_queues = 1 if not softplus_native else 4  # sim models 1 queue only
    nc = bacc.Bacc("TRN2", target_bir_lowering=False, debug=False,
                   num_swdge_queues=n_queues)

    TA = nc.dram_tensor("TA", [S_A, A_W], f32, kind="ExternalInput")
    TB = nc.dram_tensor("TB", [S_B, B_W], f32, kind="ExternalInput")
    TCS = nc.dram_tensor("TCS", [CS_ROWS, CS_W], f32, kind="ExternalInput")
    nI = sum(capsA) + capB + CS_N
    IDX = nc.dram_tensor("IDX", [128, nI // 16], i16, kind="ExternalInput")
    mA, mB, mCS = sum(capsA) // 128, capB // 128, CS_N // 128
    OA = nc.dram_tensor("OA", [128, mA * 64], f32, kind="ExternalOutput")
    OB = nc.dram_tensor("OB", [128, mB * 32], f32, kind="ExternalOutput")
    OC = nc.dram_tensor("OC", [128, 48 * 16], f32, kind="ExternalOutput")
    OS = nc.dram_tensor("OS", [128, 16 * 8], f32, kind="ExternalOutput")

    # gather segments, each <= CHUNK indices:
    # (name, src range, idx col offset, chunk cap, row width, dst slot base)
    segs = []
    o16 = 0
    slotA = 0
    for s, (r0, r1) in enumerate(A_RANGES):
        for c0, c1 in _chunks(capsA[s]):
            segs.append(("A", (r0, r1), o16, c1 - c0, A_W, slotA))
            o16 += (c1 - c0) // 16
            slotA += (c1 - c0) // 128
    slotB = 0
    for c0, c1 in _chunks(capB):
        segs.append(("B", (0, S_B), o16, c1 - c0, B_W, slotB))
        o16 += (c1 - c0) // 16
        slotB += (c1 - c0) // 128
    for c0, c1 in _chunks(CS_N):
        segs.append(("CS", (0, CS_ROWS), o16, c1 - c0, CS_W, c0 // 128))
        o16 += (c1 - c0) // 16
    # interleave A/B/CS so consecutive gathers land on different queues with
    # different sizes (smoother ring drain overlap)
    by_grp = {"A": [], "B": [], "CS": []}
    for s in segs:
        by_grp[s[0]].append(s)
    inter = []
    while any(by_grp.values()):
        for gname in ("A", "CS", "B"):
            if by_grp[gname]:
                inter.append(by_grp[gname].pop(0))
    segs = inter

    bf16 = mybir.dt.bfloat16

    def softplus_block(pool, g, gslice, d, mc, tag, out_ap, mixed):
        """out_ap[128, mc, d] = mu + softplus(rho)*eps of the gathered rows.

        mixed rows: [mu f32 d | rho bf16 d | eps bf16 d] (f32 width 2d);
        f32 rows:   [mu | rho | eps] each d f32 (in-place softplus).
        Exp and Ln share the natural_log_exp_and_others ACT table, so the
        table stays resident across all chunks (one load per kernel).
        """
        mu = g[:, gslice, 0:d]
        if mixed:
            rho = g[:, gslice, d:d + d // 2].bitcast(bf16)
            eps = g[:, gslice, d + d // 2:2 * d].bitcast(bf16)
            sp = pool.tile([128, mc, d], f32, tag=f"sp{tag}", name=f"sp{tag}")
            nc.scalar.activation(sp[:], rho, ACT.Exp)
            nc.scalar.activation(sp[:], sp[:], ACT.Ln, bias=1.0)
            nc.vector.tensor_tensor(out=sp[:], in0=sp[:], in1=eps, op=ALU.mult)
            nc.vector.tensor_tensor(out=out_ap, in0=sp[:], in1=mu, op=ALU.add)
        else:
            rho = g[:, gslice, d:2 * d]
            eps = g[:, gslice, 2 * d:3 * d]
            nc.scalar.activation(rho, rho, ACT.Exp)
            nc.scalar.activation(rho, rho, ACT.Ln, bias=1.0)
            nc.vector.tensor_tensor(out=rho, in0=rho, in1=eps, op=ALU.mult)
            nc.vector.tensor_tensor(out=out_ap, in0=rho, in1=mu, op=ALU.add)

    with tile.TileContext(nc) as tc:
        with tc.tile_pool(name="idx", bufs=1) as ipool, \
             tc.tile_pool(name="out", bufs=1) as opool, \
             tc.tile_pool(name="work", bufs=8) as wpool:
            it = ipool.tile([128, nI // 16], i16, tag="idx")
            nc.sync.dma_start(it[:], IDX.ap())
            # one num_idxs register per distinct cap: a fresh MOVE per gather
            # costs ~400ns of Pool sequencer time each
            cap_regs = {}
            for _, _, _, cap, _, _ in segs:
                if cap not in cap_regs:
                    r = nc.gpsimd.alloc_register(f"nidx{cap}")
                    nc.gpsimd.reg_mov(r, cap)
                    cap_regs[cap] = r
            OAt = opool.tile([128, mA * 64], f32, tag="OAt")
            OBt = opool.tile([128, mB * 32], f32, tag="OBt")
            OCt = opool.tile([128, 48, 16], f32, tag="OCt")
            OSt = opool.tile([128, 16, 8], f32, tag="OSt")

            for si, (name, (r0, r1), off16, cap, w, slot0) in enumerate(segs):
                mc = cap // 128
                src = (TA if name == "A" else TB if name == "B" else TCS)
                g = wpool.tile([128, mc, w], f32, tag=f"g{name}",
                               name=f"g{name}{si}")
                nc.gpsimd.dma_gather(
                    g[:], src.ap()[r0:r1, :], it[:, off16:off16 + cap // 16],
                    cap, cap_regs[cap], w, queue_num=si % n_queues)
                if name == "A":
                    softplus_block(
                        wpool, g, slice(0, mc), 64, mc, "A",
                        OAt[:, slot0 * 64:(slot0 + mc) * 64].rearrange(
                            "p (m d) -> p m d", d=64), False)
                elif name == "B":
                    softplus_block(
                        wpool, g, slice(0, mc), 32, mc, "B",
                        OBt[:, slot0 * 32:(slot0 + mc) * 32].rearrange(
                            "p (m d) -> p m d", d=32), False)
                else:
                    # slots: i = c*2048 + b_local; slot-col j = i//128 = c*16+t
                    # slot-cols [0,48) are dim-16 cols 4..6; [48,64) is col 7
                    if slot0 < 48:
                        softplus_block(wpool, g, slice(0, mc), 16, mc, "C",
                                       OCt[:, slot0:slot0 + mc, :], False)
                        nc.sync.dma_start(
                            OC.ap()[:, slot0 * 16:(slot0 + mc) * 16],
                            OCt[:, slot0:slot0 + mc, :].rearrange(
                                "p a b -> p (a b)"))
                    else:
                        softplus_block(wpool, g, slice(0, mc), 8, mc, "S",
                                       OSt[:, slot0 - 48:slot0 - 48 + mc, :],
                                       False)
                        nc.sync.dma_start(
                            OS.ap()[:, (slot0 - 48) * 8:(slot0 - 48 + mc) * 8],
                            OSt[:, slot0 - 48:slot0 - 48 + mc, :].rearrange(
                                "p a b -> p (a b)"))

            nc.sync.dma_start(OA.ap(), OAt[:])
            nc.sync.dma_start(OB.ap(), OBt[:])
    nc.compile()
    return nc


def _pack3(mu, rho, eps, w):
    """Rows [mu | rho | eps | pad] of width w (f32)."""
    n, d = mu.shape
    out = np.zeros((n, w), dtype=np.float32)
    out[:, 0:d] = mu
    out[:, d:2 * d] = rho
    out[:, 2 * d:3 * d] = eps
    return out


def _pack3_mixed(mu, rho, eps, w):
    """Rows [mu f32 d | rho bf16 d | eps bf16 d], f32 width w = 2d."""
    import ml_dtypes
    n, d = mu.shape
    assert w == 2 * d
    buf = np.empty((n, 4 * d), dtype=np.uint16)
    buf[:, 0:2 * d] = np.ascontiguousarray(mu).view(np.uint16)
    buf[:, 2 * d:3 * d] = np.ascontiguousarray(
        rho.astype(ml_dtypes.bfloat16)).view(np.uint16)
    buf[:, 3 * d:4 * d] = np.ascontiguousarray(
        eps.astype(ml_dtypes.bfloat16)).view(np.uint16)
    return buf.view(np.float32)


def _wrap16(arr):
    """int16 index array -> [128, n/16] dma_gather layout (i at [i%16, i//16],
    replicated 8x down the partition dim)."""
    n = len(arr)
    assert n % 16 == 0
    blk = arr.reshape(n // 16, 16).T  # [16, n/16]
    return np.tile(blk, (8, 1))


def _route(X, cols, shards):
    """Route (batch, col) pairs to per-column vocab-shard owners.

    Core k's table stacks [col shards]; local row of global index g in column
    j is (g % shards[j]) + sum(shards[:j]).  Returns per-core local rows (in
    slot order) and their (dest_b, dest_c)."""
    col_off = np.cumsum([0] + list(shards[:-1]))
    gid, owner, b_all, c_all = [], [], [], []
    for j, c in enumerate(cols):
        g = X[:, c].astype(np.int64)
        owner.append(g // shards[j])
        gid.append(g % shards[j] + col_off[j])
        b_all.append(np.arange(BATCH, dtype=np.int64))
        c_all.append(np.full(BATCH, c, dtype=np.int64))
    gid = np.concatenate(gid)
    owner = np.concatenate(owner)
    b_all = np.concatenate(b_all)
    c_all = np.concatenate(c_all)
    order = np.argsort(owner, kind="stable")
    counts = np.bincount(owner, minlength=N_CORES)
    locs, dests = [], []
    start = 0
    for k in range(N_CORES):
        n = int(counts[k])
        sel = order[start:start + n]
        start += n
        locs.append(gid[sel])
        dests.append((b_all[sel], c_all[sel]))
    return locs, dests


def kernel(**inputs):
    from concourse.bass_utils import run_bass_kernel_spmd

    X = np.asarray(inputs["X"])
    mus = [np.asarray(inputs[f"mu{i}"], dtype=np.float32) for i in range(8)]
    rhos = [np.asarray(inputs[f"rho{i}"], dtype=np.float32) for i in range(8)]
    epss = [np.asarray(inputs[f"eps{i}"], dtype=np.float32) for i in range(8)]

    # ---- pack tables (per-core stacked per-column shards) ----------------
    def shard_tables(cols, shards, w):
        packed = [_pack3(mus[c], rhos[c], epss[c], w) for c in cols]
        per_core = []
        for k in range(N_CORES):
            parts = []
            for j, p in enumerate(packed):
                sh = np.zeros((shards[j], w), dtype=np.float32)
                src = p[k * shards[j]:(k + 1) * shards[j]]
                sh[:len(src)] = src
                parts.append(sh)
            per_core.append(np.concatenate(parts))
        return per_core

    WA = shard_tables(A_COLS, A_SH, A_W)
    WB = shard_tables(B_COLS, B_SH, B_W)
    WCS = np.concatenate(
        [_pack3(mus[c], rhos[c], epss[c], CS_W) for c in CS_COLS])

    # ---- route A and B ---------------------------------------------------
    locsA, destA = _route(X, A_COLS, A_SH)
    locsB, destB = _route(X, B_COLS, B_SH)

    # A sub-range bucketing: per core, split local rows by 32768-row range,
    # preserving order within a bucket; caps = max over cores per bucket.
    nR = len(A_RANGES)
    bucketsA = []  # [core][bucket] -> (local_idx16, dest_b, dest_c)
    for k in range(N_CORES):
        loc = locsA[k]
        b, c = destA[k]
        sub = loc // SUB
        per = []
        for s in range(nR):
            sel = sub == s
            per.append(((loc[sel] - s * SUB).astype(np.int16), b[sel], c[sel]))
        bucketsA.append(per)
    capsA = [max(128, -(-max(len(bucketsA[k][s][0]) for k in range(N_CORES))
                        // 128) * 128) for s in range(nR)]
    capB = max(128, -(-max(len(locsB[k]) for k in range(N_CORES)) // 128) * 128)

    key = (tuple(capsA), capB, RUN_MODE)
    if key not in _nc_cache:
        _nc_cache[key] = _build_nc(list(capsA), capB,
                                   softplus_native=(RUN_MODE != "sim"))
    nc = _nc_cache[key]

    # ---- per-core inputs -------------------------------------------------
    in_maps = []
    for k in range(N_CORES):
        segs16 = []

        def add_wrapped(arr):
            # wrap each <=CHUNK gather's indices independently
            for c0, c1 in _chunks(len(arr)):
                segs16.append(_wrap16(arr[c0:c1]))

        for s in range(nR):
            arr = np.zeros(capsA[s], dtype=np.int16)
            v = bucketsA[k][s][0]
            arr[:len(v)] = v
            add_wrapped(arr)
        arrB = np.zeros(capB, dtype=np.int16)
        arrB[:len(locsB[k])] = locsB[k].astype(np.int16)
        add_wrapped(arrB)
        Xk = X[k * BPC:(k + 1) * BPC]
        arrCS = np.concatenate(
            [Xk[:, c].astype(np.int16) + CS_BASE[j]
             for j, c in enumerate(CS_COLS)])  # i = c*2048 + b_local
        add_wrapped(arrCS)
        in_maps.append({
            "TA": WA[k],
            "TB": WB[k],
            "TCS": WCS,
            "IDX": np.ascontiguousarray(np.concatenate(segs16, axis=1)),
        })

    global last_result
    if RUN_MODE == "sim":
        from concourse.bass_interp import CoreSim
        results = []
        for im in in_maps:
            sim = CoreSim(nc, trace=False)
            for kk, v in im.items():
                sim.tensor(kk)[:] = v
            sim.simulate()
            results.append({o: np.array(sim.mem_tensor(o))
                            for o in ("OA", "OB", "OC", "OS")})
        last_result = None
    else:
        res = run_bass_kernel_spmd(nc, in_maps, core_ids=list(range(N_CORES)))
        last_result = res
        results = res.results

    # ---- assemble output -------------------------------------------------
    OUT = np.empty((BATCH, DTOT), dtype=np.float32)

    def unslot(seg, cap, d):
        # device slot i -> [i % 128, i // 128]; seg is [128, (cap//128)*d]
        return seg.reshape(128, cap // 128, d).transpose(1, 0, 2).reshape(cap, d)

    for k in range(N_CORES):
        oa = results[k]["OA"]
        a_off = 0
        for s in range(nR):
            mc = capsA[s] // 128
            rows = unslot(oa[:, a_off * 64:(a_off + mc) * 64], capsA[s], 64)
            a_off += mc
            _, b, c = bucketsA[k][s]
            n = len(b)
            for col in A_COLS:
                sel = c == col
                OUT[b[sel], OFFS[col]:OFFS[col] + 64] = rows[:n][sel]
        rowsB = unslot(results[k]["OB"], capB, 32)
        b, c = destB[k]
        n = len(b)
        for col in B_COLS:
            sel = c == col
            OUT[b[sel], OFFS[col]:OFFS[col] + 32] = rowsB[:n][sel]
        # OC: [128, c(3), t(16), 16] with slot-col j = c*16 + t
        oc = results[k]["OC"].reshape(128, 3, 16, 16)
        for j, col in enumerate(CS_COLS[:3]):
            blk = oc[:, j].transpose(1, 0, 2).reshape(BPC, 16)
            OUT[k * BPC:(k + 1) * BPC, OFFS[col]:OFFS[col] + 16] = blk
        os_ = results[k]["OS"].reshape(128, 16, 8).transpose(1, 0, 2)
        OUT[k * BPC:(k + 1) * BPC, OFFS[7]:OFFS[7] + 8] = os_.reshape(BPC, 8)
    return OUT


# revision 35
# speedup vs baseline: 1.0816x; 1.0816x over previous
"""Bayesian categorical embedding lookup on 8 trn2 NeuronCores.

For each of 8 categorical columns: out = mu + softplus(rho) * eps gathered at
X[:, c]; outputs concatenated to [16384, 248] f32.

Strategy
  - Host packs each column's (mu, rho, eps) into one row-concatenated table so
    a single gathered row carries all three vectors.
  - Cols 0,1 (dim 64, 768B rows)  -> group A, vocab-sharded per column across
    the 8 cores (each core owns 1/8 of each column's rows, stacked); the host
    routes every (batch, col) pair to its owning core.
  - Cols 2,3 (dim 32, rows padded to 512B) -> group B, sharded + routed the
    same way.
  - Cols 4..7 (dims 16,16,16,8; rows padded to 256B) -> group CS, table
    replicated, batch-sharded (core k handles batch rows [2048k, 2048k+2048)).
  - Device per core: GPSIMD dma_gather (int16 indices; group A's 150002-row
    shard is gathered in 32768-row sub-ranges so indices fit int16), then
    softplus via Exp + Ln(x+1) on ACT, mult/add on DVE, compact stores.
  - Host scatters the routed rows back into the full output.

dma_gather contracts used here (see concourse/bass.py and bass_interp.py):
  - indices int16, element i at [i % 16, i // 16] of a [128, n/16] SBUF tile,
    that 16-row block replicated 8x down the partitions (one per Q7 core);
  - gathered row i lands at partition i % 128, slot i // 128 of the dst tile;
  - elem_size bytes must be a multiple of 256;
  - we pad every index segment with row 0 (always valid) so num_idxs is the
    same on all 8 cores (SPMD) and no -1 handling is needed.
"""

import numpy as np

N_CORES = 8
BATCH = 16384
BPC = BATCH // N_CORES  # 2048 batch rows per core

VOCABS = [1000000, 200000, 100000, 50000, 10000, 5000, 1000, 100]
NROWS = [v + 1 for v in VOCABS]
DIMS = [64, 64, 32, 32, 16, 16, 16, 8]
OFFS = [0, 64, 128, 160, 192, 208, 224, 240]
DTOT = 248

A_COLS, B_COLS, CS_COLS = (0, 1), (2, 3), (4, 5, 6, 7)
A_SH = [-(-NROWS[c] // N_CORES) for c in A_COLS]   # [125001, 25001]
S_A = sum(A_SH)                                    # 150002 rows per core
A_W = 192                                          # 768B f32 rows
SUB = 32768                                        # int16 sub-range size
A_RANGES = [(r, min(r + SUB, S_A)) for r in range(0, S_A, SUB)]
B_SH = [-(-NROWS[c] // N_CORES) for c in B_COLS]   # [12501, 6251]
S_B = sum(B_SH)                                    # 18752
B_W = 128                                          # 512B f32 rows
CS_BASE = [0]
for c in CS_COLS[:-1]:
    CS_BASE.append(CS_BASE[-1] + NROWS[c])
CS_ROWS = CS_BASE[-1] + NROWS[CS_COLS[-1]]         # 16104
CS_W = 64                                          # 256B rows
CS_N = BPC * len(CS_COLS)                          # 8192 gathered rows/core
CHUNK = 1024                                       # max idx per dma_gather
                                                   # (HW crashes above ~1024)


def _chunks(cap):
    return [(c0, min(c0 + CHUNK, cap)) for c0 in range(0, cap, CHUNK)]

_nc_cache = {}
last_result = None
RUN_MODE = "hw"  # "sim" runs CoreSim per core instead of hardware (debug)


def _build_nc(capsA, capB, softplus_native=True):
    """Build the SPMD Bacc program. capsA: rows gathered per A sub-range
    (each a multiple of 128, uniform across cores); capB likewise."""
    import concourse.bacc as bacc
    import concourse.mybir as mybir
    import concourse.tile as tile

    f32, i16 = mybir.dt.float32, mybir.dt.int16
    ACT = mybir.ActivationFunctionType
    ALU = mybir.AluOpType

    # Force Exp AND Ln onto the one ACT table containing both
    # (natural_log_exp_and_others): the table chooser otherwise alternates
    # exp_and_others <-> natural_log, reloading the table (1.28us) around
    # every chunk.  Table ids are dict positions, so only the function sets
    # are edited, never the order.
    if not getattr(bacc, "_ant_act_tables_patched", False):
        _orig_tables = bacc.get_activation_tables

        def _patched_tables(arch):
            t = dict(_orig_tables(arch))
            both = {mybir.ActivationFunctionType.Exp,
                    mybir.ActivationFunctionType.Ln}
            return {name: (fns if name == "natural_log_exp_and_others"
                           else fns - both)
                    for name, fns in t.items()}

        bacc.get_activation_tables = _patched_tables
        bacc._ant_act_tables_patched = True

    # 4 SWDGE queues: one qPoolDynamic ring throttles gather descriptor
    # flow to ~90 GB/s; round-robin over 4 rings measured ~1.75x faster.
    n_queues = 1 if not softplus_native else 4  # sim models 1 queue only
    nc = bacc.Bacc("TRN2", target_bir_lowering=False, debug=False,
                   num_swdge_queues=n_queues)

    TA = nc.dram_tensor("TA", [S_A, A_W], f32, kind="ExternalInput")
    TB = nc.dram_tensor("TB", [S_B, B_W], f32, kind="ExternalInput")
    TCS = nc.dram_tensor("TCS", [CS_ROWS, CS_W], f32, kind="ExternalInput")
    nI = sum(capsA) + capB + CS_N
    IDX = nc.dram_tensor("IDX", [128, nI // 16], i16, kind="ExternalInput")
    mA, mB, mCS = sum(capsA) // 128, capB // 128, CS_N // 128
    OA = nc.dram_tensor("OA", [128, mA * 64], f32, kind="ExternalOutput")
    OB = nc.dram_tensor("OB", [128, mB * 32], f32, kind="ExternalOutput")
    OC = nc.dram_tensor("OC", [128, 48 * 16], f32, kind="ExternalOutput")
    OS = nc.dram_tensor("OS", [128, 16 * 8], f32, kind="ExternalOutput")

    # gather segments, each <= CHUNK indices:
    # (name, src range, idx col offset, chunk cap, row width, dst slot base)
    segs = []
    o16 = 0
    slotA = 0
    for s, (r0, r1) in enumerate(A_RANGES):
        for c0, c1 in _chunks(capsA[s]):
            segs.append(("A", (r0, r1), o16, c1 - c0, A_W, slotA))
            o16 += (c1 - c0) // 16
            slotA += (c1 - c0) // 128
    slotB = 0
    for c0, c1 in _chunks(capB):
        segs.append(("B", (0, S_B), o16, c1 - c0, B_W, slotB))
        o16 += (c1 - c0) // 16
        slotB += (c1 - c0) // 128
    for c0, c1 in _chunks(CS_N):
        segs.append(("CS", (0, CS_ROWS), o16, c1 - c0, CS_W, c0 // 128))
        o16 += (c1 - c0) // 16

    bf16 = mybir.dt.bfloat16

    def softplus_block(pool, g, gslice, d, mc, tag, out_ap, mixed):
        """out_ap[128, mc, d] = mu + softplus(rho)*eps of the gathered rows.

        mixed rows: [mu f32 d | rho bf16 d | eps bf16 d] (f32 width 2d);
        f32 rows:   [mu | rho | eps] each d f32 (in-place softplus).
        Exp and Ln share the natural_log_exp_and_others ACT table, so the
        table stays resident across all chunks (one load per kernel).
        """
        mu = g[:, gslice, 0:d]
        if mixed:
            rho = g[:, gslice, d:d + d // 2].bitcast(bf16)
            eps = g[:, gslice, d + d // 2:2 * d].bitcast(bf16)
            sp = pool.tile([128, mc, d], f32, tag=f"sp{tag}", name=f"sp{tag}")
            nc.scalar.activation(sp[:], rho, ACT.Exp)
            nc.scalar.activation(sp[:], sp[:], ACT.Ln, bias=1.0)
            nc.vector.tensor_tensor(out=sp[:], in0=sp[:], in1=eps, op=ALU.mult)
            nc.vector.tensor_tensor(out=out_ap, in0=sp[:], in1=mu, op=ALU.add)
        else:
            rho = g[:, gslice, d:2 * d]
            eps = g[:, gslice, 2 * d:3 * d]
            nc.scalar.activation(rho, rho, ACT.Exp)
            nc.scalar.activation(rho, rho, ACT.Ln, bias=1.0)
            nc.vector.tensor_tensor(out=rho, in0=rho, in1=eps, op=ALU.mult)
            nc.vector.tensor_tensor(out=out_ap, in0=rho, in1=mu, op=ALU.add)

    with tile.TileContext(nc) as tc:
        with tc.tile_pool(name="idx", bufs=1) as ipool, \
             tc.tile_pool(name="out", bufs=1) as opool, \
             tc.tile_pool(name="work", bufs=8) as wpool:
            it = ipool.tile([128, nI // 16], i16, tag="idx")
            nc.sync.dma_start(it[:], IDX.ap())
            # one num_idxs register per distinct cap: a fresh MOVE per gather
            # costs ~400ns of Pool sequencer time each
            cap_regs = {}
            for _, _, _, cap, _, _ in segs:
                if cap not in cap_regs:
                    r = nc.gpsimd.alloc_register(f"nidx{cap}")
                    nc.gpsimd.reg_mov(r, cap)
                    cap_regs[cap] = r
            OAt = opool.tile([128, mA * 64], f32, tag="OAt")
            OBt = opool.tile([128, mB * 32], f32, tag="OBt")
            OCt = opool.tile([128, 48, 16], f32, tag="OCt")
            OSt = opool.tile([128, 16, 8], f32, tag="OSt")

            for si, (name, (r0, r1), off16, cap, w, slot0) in enumerate(segs):
                mc = cap // 128
                src = (TA if name == "A" else TB if name == "B" else TCS)
                g = wpool.tile([128, mc, w], f32, tag=f"g{name}",
                               name=f"g{name}{si}")
                nc.gpsimd.dma_gather(
                    g[:], src.ap()[r0:r1, :], it[:, off16:off16 + cap // 16],
                    cap, cap_regs[cap], w, queue_num=si % n_queues)
                if name == "A":
                    softplus_block(
                        wpool, g, slice(0, mc), 64, mc, "A",
                        OAt[:, slot0 * 64:(slot0 + mc) * 64].rearrange(
                            "p (m d) -> p m d", d=64), False)
                elif name == "B":
                    softplus_block(
                        wpool, g, slice(0, mc), 32, mc, "B",
                        OBt[:, slot0 * 32:(slot0 + mc) * 32].rearrange(
                            "p (m d) -> p m d", d=32), False)
                else:
                    # slots: i = c*2048 + b_local; slot-col j = i//128 = c*16+t
                    # slot-cols [0,48) are dim-16 cols 4..6; [48,64) is col 7
                    if slot0 < 48:
                        softplus_block(wpool, g, slice(0, mc), 16, mc, "C",
                                       OCt[:, slot0:slot0 + mc, :], False)
                    else:
                        softplus_block(wpool, g, slice(0, mc), 8, mc, "S",
                                       OSt[:, slot0 - 48:slot0 - 48 + mc, :],
                                       False)

            nc.sync.dma_start(OA.ap(), OAt[:])
            nc.sync.dma_start(OB.ap(), OBt[:])
            nc.sync.dma_start(OC.ap(), OCt[:].rearrange("p a b -> p (a b)"))
            nc.sync.dma_start(OS.ap(), OSt[:].rearrange("p a b -> p (a b)"))
    nc.compile()
    return nc


def _pack3(mu, rho, eps, w):
    """Rows [mu | rho | eps | pad] of width w (f32)."""
    n, d = mu.shape
    out = np.zeros((n, w), dtype=np.float32)
    out[:, 0:d] = mu
    out[:, d:2 * d] = rho
    out[:, 2 * d:3 * d] = eps
    return out


def _pack3_mixed(mu, rho, eps, w):
    """Rows [mu f32 d | rho bf16 d | eps bf16 d], f32 width w = 2d."""
    import ml_dtypes
    n, d = mu.shape
    assert w == 2 * d
    buf = np.empty((n, 4 * d), dtype=np.uint16)
    buf[:, 0:2 * d] = np.ascontiguousarray(mu).view(np.uint16)
    buf[:, 2 * d:3 * d] = np.ascontiguousarray(
        rho.astype(ml_dtypes.bfloat16)).view(np.uint16)
    buf[:, 3 * d:4 * d] = np.ascontiguousarray(
        eps.astype(ml_dtypes.bfloat16)).view(np.uint16)
    return buf.view(np.float32)


def _wrap16(arr):
    """int16 index array -> [128, n/16] dma_gather layout (i at [i%16, i//16],
    replicated 8x down the partition dim)."""
    n = len(arr)
    assert n % 16 == 0
    blk = arr.reshape(n // 16, 16).T  # [16, n/16]
    return np.tile(blk, (8, 1))


def _route(X, cols, shards):
    """Route (batch, col) pairs to per-column vocab-shard owners.

    Core k's table stacks [col shards]; local row of global index g in column
    j is (g % shards[j]) + sum(shards[:j]).  Returns per-core local rows (in
    slot order) and their (dest_b, dest_c)."""
    col_off = np.cumsum([0] + list(shards[:-1]))
    gid, owner, b_all, c_all = [], [], [], []
    for j, c in enumerate(cols):
        g = X[:, c].astype(np.int64)
        owner.append(g // shards[j])
        gid.append(g % shards[j] + col_off[j])
        b_all.append(np.arange(BATCH, dtype=np.int64))
        c_all.append(np.full(BATCH, c, dtype=np.int64))
    gid = np.concatenate(gid)
    owner = np.concatenate(owner)
    b_all = np.concatenate(b_all)
    c_all = np.concatenate(c_all)
    order = np.argsort(owner, kind="stable")
    counts = np.bincount(owner, minlength=N_CORES)
    locs, dests = [], []
    start = 0
    for k in range(N_CORES):
        n = int(counts[k])
        sel = order[start:start + n]
        start += n
        locs.append(gid[sel])
        dests.append((b_all[sel], c_all[sel]))
    return locs, dests


def kernel(**inputs):
    from concourse.bass_utils import run_bass_kernel_spmd

    X = np.asarray(inputs["X"])
    mus = [np.asarray(inputs[f"mu{i}"], dtype=np.float32) for i in range(8)]
    rhos = [np.asarray(inputs[f"rho{i}"], dtype=np.float32) for i in range(8)]
    epss = [np.asarray(inputs[f"eps{i}"], dtype=np.float32) for i in range(8)]

    # ---- pack tables (per-core stacked per-column shards) ----------------
    def shard_tables(cols, shards, w):
        packed = [_pack3(mus[c], rhos[c], epss[c], w) for c in cols]
        per_core = []
        for k in range(N_CORES):
            parts = []
            for j, p in enumerate(packed):
                sh = np.zeros((shards[j], w), dtype=np.float32)
                src = p[k * shards[j]:(k + 1) * shards[j]]
                sh[:len(src)] = src
                parts.append(sh)
            per_core.append(np.concatenate(parts))
        return per_core

    WA = shard_tables(A_COLS, A_SH, A_W)
    WB = shard_tables(B_COLS, B_SH, B_W)
    WCS = np.concatenate(
        [_pack3(mus[c], rhos[c], epss[c], CS_W) for c in CS_COLS])

    # ---- route A and B ---------------------------------------------------
    locsA, destA = _route(X, A_COLS, A_SH)
    locsB, destB = _route(X, B_COLS, B_SH)

    # A sub-range bucketing: per core, split local rows by 32768-row range,
    # preserving order within a bucket; caps = max over cores per bucket.
    nR = len(A_RANGES)
    bucketsA = []  # [core][bucket] -> (local_idx16, dest_b, dest_c)
    for k in range(N_CORES):
        loc = locsA[k]
        b, c = destA[k]
        sub = loc // SUB
        per = []
        for s in range(nR):
            sel = sub == s
            per.append(((loc[sel] - s * SUB).astype(np.int16), b[sel], c[sel]))
        bucketsA.append(per)
    capsA = [max(128, -(-max(len(bucketsA[k][s][0]) for k in range(N_CORES))
                        // 128) * 128) for s in range(nR)]
    capB = max(128, -(-max(len(locsB[k]) for k in range(N_CORES)) // 128) * 128)

    key = (tuple(capsA), capB, RUN_MODE)
    if key not in _nc_cache:
        _nc_cache[key] = _build_nc(list(capsA), capB,
                                   softplus_native=(RUN_MODE != "sim"))
    nc = _nc_cache[key]

    # ---- per-core inputs -------------------------------------------------
    in_maps = []
    for k in range(N_CORES):
        segs16 = []

        def add_wrapped(arr):
            # wrap each <=CHUNK gather's indices independently
            for c0, c1 in _chunks(len(arr)):
                segs16.append(_wrap16(arr[c0:c1]))

        for s in range(nR):
            arr = np.zeros(capsA[s], dtype=np.int16)
            v = bucketsA[k][s][0]
            arr[:len(v)] = v
            add_wrapped(arr)
        arrB = np.zeros(capB, dtype=np.int16)
        arrB[:len(locsB[k])] = locsB[k].astype(np.int16)
        add_wrapped(arrB)
        Xk = X[k * BPC:(k + 1) * BPC]
        arrCS = np.concatenate(
            [Xk[:, c].astype(np.int16) + CS_BASE[j]
             for j, c in enumerate(CS_COLS)])  # i = c*2048 + b_local
        add_wrapped(arrCS)
        in_maps.append({
            "TA": WA[k],
            "TB": WB[k],
            "TCS": WCS,
            "IDX": np.ascontiguousarray(np.concatenate(segs16, axis=1)),
        })

    global last_result
    if RUN_MODE == "sim":
        from concourse.bass_interp import CoreSim
        results = []
        for im in in_maps:
            sim = CoreSim(nc, trace=False)
            for kk, v in im.items():
                sim.tensor(kk)[:] = v
            sim.simulate()
            results.append({o: np.array(sim.mem_tensor(o))
                            for o in ("OA", "OB", "OC", "OS")})
        last_result = None
    else:
        res = run_bass_kernel_spmd(nc, in_maps, core_ids=list(range(N_CORES)))
        last_result = res
        results = res.results

    # ---- assemble output -------------------------------------------------
    OUT = np.empty((BATCH, DTOT), dtype=np.float32)

    def unslot(seg, cap, d):
        # device slot i -> [i % 128, i // 128]; seg is [128, (cap//128)*d]
        return seg.reshape(128, cap // 128, d).transpose(1, 0, 2).reshape(cap, d)

    for k in range(N_CORES):
        oa = results[k]["OA"]
        a_off = 0
        for s in range(nR):
            mc = capsA[s] // 128
            rows = unslot(oa[:, a_off * 64:(a_off + mc) * 64], capsA[s], 64)
            a_off += mc
            _, b, c = bucketsA[k][s]
            n = len(b)
            for col in A_COLS:
                sel = c == col
                OUT[b[sel], OFFS[col]:OFFS[col] + 64] = rows[:n][sel]
        rowsB = unslot(results[k]["OB"], capB, 32)
        b, c = destB[k]
        n = len(b)
        for col in B_COLS:
            sel = c == col
            OUT[b[sel], OFFS[col]:OFFS[col] + 32] = rowsB[:n][sel]
        # OC: [128, c(3), t(16), 16] with slot-col j = c*16 + t
        oc = results[k]["OC"].reshape(128, 3, 16, 16)
        for j, col in enumerate(CS_COLS[:3]):
            blk = oc[:, j].transpose(1, 0, 2).reshape(BPC, 16)
            OUT[k * BPC:(k + 1) * BPC, OFFS[col]:OFFS[col] + 16] = blk
        os_ = results[k]["OS"].reshape(128, 16, 8).transpose(1, 0, 2)
        OUT[k * BPC:(k + 1) * BPC, OFFS[7]:OFFS[7] + 8] = os_.reshape(BPC, 8)
    return OUT
